# revision 1
# baseline (speedup 1.0000x reference)
"""Trainium2 Bass kernel for nn_AbstractODEDecoder.

Reference computation:
  - ODE dL/dt = MLP_tanh([L, z_rest, t]) integrated over t in [0,1]
    (dopri5 in the reference), latents needed at the 128 grid times.
  - Decode: relu MLP on [t, L(t), z_rest] at each of the 128 grid times.

Integration here: fixed-step Heun on 64 double-width steps; the odd grid
points come from cubic-Hermite interpolation using the k1 RHS evaluations
Heun computes anyway (L(mid) = (L_m+L_{m+1})/2 + (k1_m-k1_{m+1})*h/8, with
k1_{m+1} shared with the next step's first stage). Integration + interp
error is ~3e-6, far below the float32r matmul noise (~2.6e-4 end to end,
validated against the dopri5 reference).

Sharding: data-parallel over batch. 2048 rows -> 8 cores x 256 rows.

Layout: activations are feature-major ([features on partitions, batch on
free dim]) so weight matrices serve directly as matmul lhsT operands. The
shared time grid lets the time-dependent bias terms (t*W1[128,:]+b1,
t*D1[0,:]+c1) enter through K=1 rank-1 matmuls (table row x ones) into
the same PSUM accumulation, keeping each tanh/relu one wide activation
op. The final decode layer flips to batch-major (activation tiles as
lhsT) so output DMA is contiguous.

Heun scale factors fold into pre-scaled W3 copies (x h, x h/2) so each
stage's state update is a single vector op past the matmuls. The
integrator state is kept exact in fp32 ping-pong tiles; the
f32r-quantized copies seen by matmuls/decode are written separately
(f32r rounds matmul operands to 12 mantissa bits; quantizing the state
itself would random-walk ~1e-3 over the integration).
"""

import numpy as np

B, P = 2048, 128
ZDIM, HDIM, LDIM = 128, 512, 64
NCORES = 8
BC = B // NCORES          # batch rows per core (256)
NB = BC // 128            # batch partition tiles (2)
NODE = P // 2             # Heun integration steps (64)
H2 = 2.0 / P              # ODE step size

_cache = {}


def _build(mm_dtype_name="float32r", npairs=NODE, repeat=1,
           with_b2=False, with_b3=False, with_c2=False, with_c3=False):
    import contextlib
    import concourse.bass as bass
    import concourse.mybir as mybir
    import concourse.tile as tile
    from concourse import bacc
    from concourse.masks import make_identity

    f32 = mybir.dt.float32
    mdt = getattr(mybir.dt, mm_dtype_name)
    AF = mybir.ActivationFunctionType
    ALU = mybir.AluOpType

    nc = bacc.Bacc("TRN2", target_bir_lowering=False, debug=False,
                   num_devices=NCORES)

    # ---- DRAM I/O ----
    zin = nc.dram_tensor("zin", [BC, ZDIM], f32, kind="ExternalInput")
    w1 = nc.dram_tensor("w1", [ZDIM, HDIM], mdt, kind="ExternalInput")
    w2 = nc.dram_tensor("w2", [HDIM, HDIM], mdt, kind="ExternalInput")
    w3h = nc.dram_tensor("w3h", [HDIM, LDIM], mdt, kind="ExternalInput")
    w3h2 = nc.dram_tensor("w3h2", [HDIM, LDIM], mdt, kind="ExternalInput")
    d1 = nc.dram_tensor("d1", [ZDIM, HDIM], mdt, kind="ExternalInput")
    d2 = nc.dram_tensor("d2", [HDIM, HDIM], mdt, kind="ExternalInput")
    d3 = nc.dram_tensor("d3", [HDIM, HDIM], mdt, kind="ExternalInput")
    # time-bias rows: tbr[k] = b1 + (k/P)*W1[128,:], k = 0..P
    tbr = nc.dram_tensor("tbr", [P + 1, HDIM], mdt, kind="ExternalInput")
    # decode-bias rows: cbr[i] = c1 + t_{i+1}*D1[0,:], i = 0..P-1
    cbr = nc.dram_tensor("cbr", [P, HDIM], mdt, kind="ExternalInput")
    b2r = nc.dram_tensor("b2r", [1, HDIM], mdt, kind="ExternalInput")
    b3hr = nc.dram_tensor("b3hr", [2, LDIM], mdt, kind="ExternalInput")
    c2r = nc.dram_tensor("c2r", [1, HDIM], mdt, kind="ExternalInput")
    c3r_d = nc.dram_tensor("c3r_d", [1, HDIM], mdt, kind="ExternalInput")
    onesd = nc.dram_tensor("onesd", [1, 2 * BC], mdt, kind="ExternalInput")
    out = nc.dram_tensor("out", [BC, P, HDIM], f32, kind="ExternalOutput")

    with tile.TileContext(nc) as tc:
        with tc.tile_pool(name="const", bufs=1) as const, \
             tc.tile_pool(name="act", bufs=3) as act, \
             tc.tile_pool(name="dec", bufs=2) as dec, \
             tc.tile_pool(name="small", bufs=4) as small, \
             tc.tile_pool(name="rows", bufs=6) as rows, \
             tc.tile_pool(name="outp", bufs=4) as outp, \
             tc.tile_pool(name="ph", bufs=2, space="PSUM") as ph, \
             tc.tile_pool(name="pk", bufs=2, space="PSUM") as pk, \
             tc.tile_pool(name="pd", bufs=2, space="PSUM") as pd:

            # ---- load weights / tables ----
            w1t = const.tile([ZDIM, HDIM], mdt)
            nc.sync.dma_start(out=w1t, in_=w1[:, :])
            w2t = [const.tile([128, HDIM], mdt, name=f"w2t{k}") for k in range(4)]
            for k in range(4):
                nc.sync.dma_start(out=w2t[k], in_=w2[k * 128:(k + 1) * 128, :])
            w3ht = [const.tile([128, LDIM], mdt, name=f"w3ht{k}") for k in range(4)]
            w3h2t = [const.tile([128, LDIM], mdt, name=f"w3h2t{k}")
                     for k in range(4)]
            for k in range(4):
                nc.sync.dma_start(out=w3ht[k], in_=w3h[k * 128:(k + 1) * 128, :])
                nc.sync.dma_start(out=w3h2t[k], in_=w3h2[k * 128:(k + 1) * 128, :])
            d1t = const.tile([ZDIM, HDIM], mdt)
            nc.sync.dma_start(out=d1t, in_=d1[:, :])
            d2t = [const.tile([128, HDIM], mdt, name=f"d2t{k}") for k in range(4)]
            for k in range(4):
                nc.sync.dma_start(out=d2t[k], in_=d2[k * 128:(k + 1) * 128, :])
            d3t = [const.tile([128, HDIM], mdt, name=f"d3t{k}") for k in range(4)]
            for k in range(4):
                nc.sync.dma_start(out=d3t[k], in_=d3[k * 128:(k + 1) * 128, :])
            b2rt = const.tile([1, HDIM], mdt)
            nc.sync.dma_start(out=b2rt, in_=b2r[:, :])
            b3hrt = [const.tile([1, LDIM], mdt, name=f"b3hrt{s_}")
                     for s_ in range(2)]
            for s_ in range(2):
                nc.sync.dma_start(out=b3hrt[s_], in_=b3hr[s_:s_ + 1, :])
            c2rt = const.tile([1, HDIM], mdt)
            nc.sync.dma_start(out=c2rt, in_=c2r[:, :])
            c3rt = const.tile([1, HDIM], mdt)
            nc.sync.dma_start(out=c3rt, in_=c3r_d[:, :])
            onest = const.tile([1, 2 * BC], mdt)
            nc.sync.dma_start(out=onest, in_=onesd[:, :])
            ident = const.tile([128, 128], f32)
            make_identity(nc, ident)

            # ---- state buffers ----
            # vto: ODE stage-input buffer, alternating 256-wide halves.
            # vtd[p]: decode input buffers (pair parity p):
            #   half0 = endpoint state, half1 = interpolated midpoint.
            vto = const.tile([ZDIM, 2 * BC], mdt)
            vtd = [const.tile([ZDIM, 2 * BC], mdt, name=f"vtd{p}")
                   for p in range(2)]
            # exact fp32 state, ping-pong: lf[m%2] = L_m
            lf = [const.tile([LDIM, BC], f32, name=f"lf{p}") for p in range(2)]
            for nb in range(NB):
                zb = small.tile([128, ZDIM], f32, tag="zb")
                nc.sync.dma_start(out=zb, in_=zin[nb * 128:(nb + 1) * 128, :])
                ztp = pd.tile([ZDIM, 128], f32, tag="pdec")
                nc.tensor.transpose(ztp, zb, ident)
                zts = small.tile([ZDIM, 128], f32, tag="zts")
                nc.vector.tensor_copy(zts, ztp)
                nc.vector.tensor_copy(
                    vto[0:LDIM, nb * 128:(nb + 1) * 128], zts[0:LDIM, :])
                nc.vector.tensor_copy(
                    lf[0][:, nb * 128:(nb + 1) * 128], zts[0:LDIM, :])
                for hf in range(2):
                    nc.vector.tensor_copy(
                        vto[LDIM:ZDIM,
                            hf * BC + nb * 128: hf * BC + (nb + 1) * 128],
                        zts[LDIM:ZDIM, :])
                    for p in range(2):
                        nc.vector.tensor_copy(
                            vtd[p][LDIM:ZDIM,
                                   hf * BC + nb * 128: hf * BC + (nb + 1) * 128],
                            zts[LDIM:ZDIM, :])

            # ---- RHS eval: k (pre-scaled) lands in psum (64,BC) ----
            # scale=0 -> W3*h (stage 1), scale=1 -> W3*h/2 (stage 2)
            def rhs_eval(col, trow, scale, sname):
                h1p = ph.tile([128, 4 * BC], f32, tag="ph", name=f"h1p_{sname}")
                for j in range(4):
                    nc.tensor.matmul(
                        h1p[:, j * BC:(j + 1) * BC],
                        trow[0:1, j * 128:(j + 1) * 128], onest[:, 0:BC],
                        start=True, stop=False)
                    nc.tensor.matmul(
                        h1p[:, j * BC:(j + 1) * BC],
                        w1t[:, j * 128:(j + 1) * 128],
                        vto[:, col:col + BC], start=False, stop=True)
                h1s = act.tile([128, 4 * BC], mdt, tag="hs", name=f"h1s_{sname}")
                nc.scalar.activation(h1s, h1p, AF.Tanh)
                h2p = ph.tile([128, 4 * BC], f32, tag="ph", name=f"h2p_{sname}")
                for j in range(4):
                    if with_b2:
                        nc.tensor.matmul(
                            h2p[:, j * BC:(j + 1) * BC],
                            b2rt[0:1, j * 128:(j + 1) * 128], onest[:, 0:BC],
                            start=True, stop=False)
                    for k in range(4):
                        nc.tensor.matmul(
                            h2p[:, j * BC:(j + 1) * BC],
                            w2t[k][:, j * 128:(j + 1) * 128],
                            h1s[:, k * BC:(k + 1) * BC],
                            start=(k == 0 and not with_b2), stop=(k == 3))
                h2s = act.tile([128, 4 * BC], mdt, tag="hs", name=f"h2s_{sname}")
                nc.scalar.activation(h2s, h2p, AF.Tanh)
                kp = pk.tile([LDIM, BC], f32, tag="pk", name=f"kp_{sname}")
                w3 = w3ht if scale == 0 else w3h2t
                if with_b3:
                    nc.tensor.matmul(kp, b3hrt[scale], onest[:, 0:BC],
                                     start=True, stop=False)
                for k in range(4):
                    nc.tensor.matmul(kp, w3[k], h2s[:, k * BC:(k + 1) * BC],
                                     start=(k == 0 and not with_b3),
                                     stop=(k == 3))
                return kp

            # ---- decode pair pr: outputs idx 2pr (half1) and 2pr+1 (half0) ----
            def make_decode_chunks(pr, b, crow):
                g1s = dec.tile([128, 4 * 2 * BC], mdt, tag="gs",
                               name=f"g1s_{pr}")
                g2s = dec.tile([128, 4 * 2 * BC], mdt, tag="gs",
                               name=f"g2s_{pr}")

                def chunk1():
                    for j in range(4):
                        g1p = pd.tile([128, 2 * BC], f32, tag="pdec",
                                      name=f"g1p_{pr}_{j}")
                        nc.tensor.matmul(
                            g1p[:, 0:BC],
                            crow[1][0:1, j * 128:(j + 1) * 128], onest[:, 0:BC],
                            start=True, stop=False)
                        nc.tensor.matmul(g1p[:, 0:BC],
                                         d1t[:, j * 128:(j + 1) * 128],
                                         b[:, 0:BC], start=False, stop=True)
                        nc.tensor.matmul(
                            g1p[:, BC:2 * BC],
                            crow[0][0:1, j * 128:(j + 1) * 128], onest[:, 0:BC],
                            start=True, stop=False)
                        nc.tensor.matmul(g1p[:, BC:2 * BC],
                                         d1t[:, j * 128:(j + 1) * 128],
                                         b[:, BC:2 * BC], start=False, stop=True)
                        nc.scalar.activation(
                            g1s[:, j * 2 * BC:(j + 1) * 2 * BC], g1p, AF.Relu)

                def chunk2():
                    for j in range(4):
                        g2p = pd.tile([128, 2 * BC], f32, tag="pdec",
                                      name=f"g2p_{pr}_{j}")
                        if with_c2:
                            nc.tensor.matmul(
                                g2p, c2rt[0:1, j * 128:(j + 1) * 128],
                                onest[:, 0:2 * BC], start=True, stop=False)
                        for k in range(4):
                            nc.tensor.matmul(
                                g2p, d2t[k][:, j * 128:(j + 1) * 128],
                                g1s[:, k * 2 * BC:(k + 1) * 2 * BC],
                                start=(k == 0 and not with_c2), stop=(k == 3))
                        nc.vector.tensor_scalar(
                            g2s[:, j * 2 * BC:(j + 1) * 2 * BC], g2p,
                            0.0, None, op0=ALU.max)

                def out_chunk(mts):
                    def go():
                        for mt in mts:
                            op = pd.tile([128, HDIM], f32, tag="pdec",
                                         name=f"op_{pr}_{mt}")
                            if with_c3:
                                nc.tensor.matmul(
                                    op, onest[:, 0:128],
                                    c3rt, start=True, stop=False)
                            for k in range(4):
                                nc.tensor.matmul(
                                    op,
                                    g2s[:, k * 2 * BC + mt * 128:
                                        k * 2 * BC + (mt + 1) * 128],
                                    d3t[k], start=(k == 0 and not with_c3),
                                    stop=(k == 3))
                            os = outp.tile([128, HDIM], f32, tag="os",
                                           name=f"os_{pr}_{mt}")
                            nc.vector.tensor_scalar(os, op, 0.0, None,
                                                    op0=ALU.max)
                            tidx = 2 * pr + 1 if mt < 2 else 2 * pr
                            rb = (mt % 2) * 128
                            nc.sync.dma_start(out=out[rb:rb + 128, tidx, :],
                                              in_=os)
                    return go

                return [chunk1, chunk2, out_chunk([0, 1]), out_chunk([2, 3])]

            # ---- Heun steps over double-width intervals ----
            rep_ctx = (tc.For_i(0, repeat, 1) if repeat > 1
                       else contextlib.nullcontext())
            with rep_ctx:
              pending = []
              wprev = None
              u2 = None
              for m in range(npairs + 1):
                last = (m == npairs)
                oA, oB = (m % 2) * BC, ((m + 1) % 2) * BC
                trow1 = rows.tile([1, HDIM], mdt, tag="trow1", name=f"tr1_{m}")
                nc.sync.dma_start(out=trow1, in_=tbr[2 * m:2 * m + 1, :])

                # stage 1 at tau_m: kp1 = h*k1  (also f_m for the interp)
                kp1 = rhs_eval(oA, trow1, 0, f"s1_{m}")
                w = small.tile([LDIM, BC], f32, tag="w", name=f"w_{m}")
                nc.vector.tensor_scalar(w, kp1, 0.5, None, op0=ALU.mult)
                if not last:
                    # Ltmp -> ODE colB: single TT on the critical path
                    nc.vector.tensor_tensor(vto[0:LDIM, oB:oB + BC],
                                            lf[m % 2], kp1, op=ALU.add)
                    u2 = small.tile([LDIM, BC], f32, tag="u2", name=f"u2_{m}")
                    nc.vector.tensor_tensor(u2, lf[m % 2], w, op=ALU.add)

                if m > 0:
                    # midpoint of interval m-1 -> vtd[(m-1)%2] half1:
                    # (L_{m-1}+L_m)/2 + (kp1_{m-1}-kp1_m)/8
                    e1 = small.tile([LDIM, BC], f32, tag="e1", name=f"e1_{m}")
                    nc.vector.tensor_tensor(e1, lf[(m - 1) % 2], lf[m % 2],
                                            op=ALU.add)
                    e2 = small.tile([LDIM, BC], f32, tag="e2", name=f"e2_{m}")
                    nc.vector.tensor_tensor(e2, wprev, w, op=ALU.subtract)
                    e3 = small.tile([LDIM, BC], f32, tag="e3", name=f"e3_{m}")
                    nc.vector.tensor_scalar(e3, e2, 0.5, None, op0=ALU.mult)
                    e4 = small.tile([LDIM, BC], f32, tag="e4", name=f"e4_{m}")
                    nc.vector.tensor_tensor(e4, e1, e3, op=ALU.add)
                    nc.vector.tensor_scalar(
                        vtd[(m - 1) % 2][0:LDIM, BC:2 * BC], e4, 0.5, None,
                        op0=ALU.mult)
                    # previous pair's decode can start now
                    for ch in pending:
                        ch()
                    cr0 = rows.tile([1, HDIM], mdt, tag="crow0",
                                    name=f"cr0_{m}")
                    nc.sync.dma_start(out=cr0,
                                      in_=cbr[2 * (m - 1):2 * (m - 1) + 1, :])
                    cr1 = rows.tile([1, HDIM], mdt, tag="crow1",
                                    name=f"cr1_{m}")
                    nc.sync.dma_start(out=cr1, in_=cbr[2 * m - 1:2 * m, :])
                    pending = make_decode_chunks(m - 1, vtd[(m - 1) % 2],
                                                 (cr0, cr1))
                wprev = w
                if last:
                    break

                if pending:
                    pending.pop(0)()
                # stage 2 at tau_{m+1}
                trow2 = rows.tile([1, HDIM], mdt, tag="trow2", name=f"tr2_{m}")
                nc.sync.dma_start(out=trow2, in_=tbr[2 * m + 2:2 * m + 3, :])
                kp2 = rhs_eval(oB, trow2, 1, f"s2_{m}")
                # L_{m+1}: ODE colB first (critical), then decode buffer, lf
                nc.vector.tensor_tensor(vto[0:LDIM, oB:oB + BC], u2, kp2,
                                        op=ALU.add)
                nc.vector.tensor_tensor(vtd[m % 2][0:LDIM, 0:BC], u2, kp2,
                                        op=ALU.add)
                nc.vector.tensor_tensor(lf[(m + 1) % 2], u2, kp2, op=ALU.add)
                if pending:
                    pending.pop(0)()
              # flush the final pair's decode
              for ch in pending:
                  ch()

    nc.compile()
    return nc


def _prepare(inputs):
    """Host-side prep: per-core input dicts (small O(weights) transforms)."""
    x = np.asarray(inputs["x"], np.float32)
    z = np.ascontiguousarray(np.asarray(inputs["z"], np.float32))
    W1 = np.asarray(inputs["W1"], np.float32)
    b1 = np.asarray(inputs["b1"], np.float32)
    W2 = np.ascontiguousarray(np.asarray(inputs["W2"], np.float32))
    b2 = np.asarray(inputs["b2"], np.float32)
    W3 = np.asarray(inputs["W3"], np.float32)
    b3 = np.asarray(inputs["b3"], np.float32)
    D1 = np.asarray(inputs["D1"], np.float32)
    c1 = np.asarray(inputs["c1"], np.float32)
    D2 = np.ascontiguousarray(np.asarray(inputs["D2"], np.float32))
    c2 = np.asarray(inputs["c2"], np.float32)
    D3 = np.ascontiguousarray(np.asarray(inputs["D3"], np.float32))
    c3 = np.asarray(inputs["c3"], np.float32)

    grid = x[0, :, 0]                      # (P,) shared time grid
    tall = np.concatenate([[np.float32(0.0)], grid])  # (P+1,) = i/P

    h = np.float32(H2)
    shared = {
        "w1": np.ascontiguousarray(W1[:128]),
        "w2": W2,
        "w3h": np.ascontiguousarray(W3 * h),
        "w3h2": np.ascontiguousarray(W3 * (h / 2)),
        "d1": np.ascontiguousarray(D1[1:129]),
        "d2": D2, "d3": D3,
        "tbr": np.ascontiguousarray(b1[None, :] + tall[:, None] * W1[128][None, :]),
        "cbr": np.ascontiguousarray(c1[None, :] + grid[:, None] * D1[0][None, :]),
        "b2r": np.ascontiguousarray(b2[None, :]),
        "b3hr": np.ascontiguousarray(np.stack([b3 * h, b3 * (h / 2)], axis=0)),
        "c2r": np.ascontiguousarray(c2[None, :]),
        "c3r_d": np.ascontiguousarray(c3[None, :]),
        "onesd": np.ones((1, 2 * BC), np.float32),
    }
    flags = {
        "with_b2": bool(np.any(b2 != 0)),
        "with_b3": bool(np.any(b3 != 0)),
        "with_c2": bool(np.any(c2 != 0)),
        "with_c3": bool(np.any(c3 != 0)),
    }
    in_maps = []
    for c in range(NCORES):
        m = dict(shared)
        m["zin"] = np.ascontiguousarray(z[c * BC:(c + 1) * BC])
        in_maps.append(m)
    return in_maps, flags


def kernel(**inputs):
    from concourse.bass_utils import run_bass_kernel_spmd

    in_maps, flags = _prepare(inputs)
    key = tuple(sorted(flags.items()))
    if key not in _cache:
        _cache[key] = _build(**flags)
    nc = _cache[key]
    res = run_bass_kernel_spmd(nc, in_maps, core_ids=list(range(NCORES)))
    return np.concatenate([r["out"] for r in res.results], axis=0)

